# revision 29
# baseline (speedup 1.0000x reference)
"""3-layer GCN encoder (PyG GCNConv semantics) on 8 Trainium2 NeuronCores.

Strategy (dst-sharded message passing):
  - Nodes are 1D-partitioned across the 8 cores (node n -> core n // (N/8)).
  - Per layer l:  z = (dinv * x) @ W_l   computed shard-wise in fp32 on PE,
    rounded to bf16, AllGathered into a full [N, D] bf16 table in DRAM.
    (norm factorizes: out[i] = dinv[i] * sum_e dinv[src_e] * (x@W)[src_e],
     so no per-edge scaling is needed.)
  - Each core owns the edges whose dst lands in its shard. Edge messages are
    fetched with GPSIMD dma_gather (HBM->SBUF, 256B rows by int16 index; srcs
    >= 32768 are gathered through a +32768 base view).  Aggregation is done
    on the PE: for each 128-token slot, a one-hot matrix H[t, dstlocal_t]
    (built on DVE via iota==dstlocal) is matmul'ed with the gathered tokens
    and accumulated in PSUM per 128-node block -> exact fp32 segment sum.
  - Epilogue: e = dinv * agg + b (fp32), written out per shard; running
    total and next-layer x stay SBUF-resident.

Host side: edge bucketing by (core, block, low/high src), per-block padding
to 128-token slots with (src=0, dstlocal=-1) dummies (H row is all-zero so
they contribute nothing), index/one-hot-scalar array packing, and final
unshard (concat + slice).
"""

import math

import numpy as np
import ml_dtypes

from concourse import bass, bacc, mybir, library_config
import concourse.tile as tile

BF16 = ml_dtypes.bfloat16
P = 128
LOW_LIM = 32768
F32 = mybir.dt.float32
BF = mybir.dt.bfloat16
I16 = mybir.dt.int16


# ----------------------------------------------------------------------------
# host-side preprocessing
# ----------------------------------------------------------------------------

class Plan:
    """Static (core-invariant) program structure + per-core packed arrays."""
    pass


def build_plan(edge_index, n, n_cores, group_blocks=7, max_chunk_slots=80):
    src_e = np.asarray(edge_index[0], dtype=np.int64)
    dst_e = np.asarray(edge_index[1], dtype=np.int64)
    # self-loops are handled analytically on-device (dinv * z term), so the
    # token stream only carries the real edges; degree still counts them.
    src = src_e
    dst = dst_e

    deg = (np.bincount(dst, minlength=n) + 1).astype(np.float64)
    dinv = (1.0 / np.sqrt(deg)).astype(np.float32)

    assert n % n_cores == 0
    npc = n // n_cores
    nb = math.ceil(npc / P)
    npc_pad = nb * P

    # z-table order: per-layer AllGather is split in two so the big first
    # part can overlap the previous layer's gathers. Blocks [0, nbA) of every
    # core land in table half A (rank-major), the last group's blocks in
    # half B. Gather indices use this table position, not the node id.
    nbA = max(nb - group_blocks, 1) if nb > group_blocks else nb
    hA = min(nbA * P, npc)
    hB = npc - hA

    core = dst // npc
    blk = (dst % npc) // P
    dstl = (dst % npc) % P
    src_core = src // npc
    src_loc = src % npc
    tpos = np.where(
        src_loc < hA,
        src_core * hA + src_loc,
        n_cores * hA + src_core * hB + (src_loc - hA),
    )
    low = tpos < LOW_LIM
    gidx_val = np.where(low, tpos, tpos - LOW_LIM)

    # stable-sort edges by (core, blk, high?) -> contiguous groups
    key = (core * nb + blk) * 2 + (~low).astype(np.int64)
    order = np.argsort(key, kind="stable")
    key_s = key[order]
    gidx_s = gidx_val[order]
    dstl_s = dstl[order]

    ngroups_keys = n_cores * nb * 2
    cnt = np.bincount(key_s, minlength=ngroups_keys)
    cnt_lo = cnt[0::2].reshape(n_cores, nb)
    cnt_hi = cnt[1::2].reshape(n_cores, nb)

    # slots per block index (max over cores) -> core-invariant structure
    S_lo = np.maximum(1, np.ceil(cnt_lo / P).astype(np.int64).max(axis=0))
    S_hi = np.maximum(1, np.ceil(cnt_hi / P).astype(np.int64).max(axis=0))

    # slot layout: per group of blocks: [lo slots of blocks][hi slots of blocks]
    G = group_blocks
    ngrp = math.ceil(nb / G)
    blk_lo0 = np.zeros(nb, dtype=np.int64)
    blk_hi0 = np.zeros(nb, dtype=np.int64)
    chunks = []  # (slot0, nslots, is_high, [block ids])
    s_cur = 0
    for g in range(ngrp):
        rs = list(range(g * G, min(nb, (g + 1) * G)))
        c0 = s_cur
        for r in rs:
            blk_lo0[r] = s_cur
            s_cur += S_lo[r]
        # split long runs into <= max_chunk_slots chunks
        chunks.append((c0, s_cur - c0, False, rs))
        c0 = s_cur
        for r in rs:
            blk_hi0[r] = s_cur
            s_cur += S_hi[r]
        chunks.append((c0, s_cur - c0, True, rs))
    S_total = s_cur

    # split oversize chunks (keeps gather tiles bounded)
    chunks2 = []
    for (c0, ns, hi, rs) in chunks:
        while ns > max_chunk_slots:
            chunks2.append((c0, max_chunk_slots, hi, rs))
            c0 += max_chunk_slots
            ns -= max_chunk_slots
        if ns > 0:
            chunks2.append((c0, ns, hi, rs))
    chunks = chunks2

    # pack tokens: position of edge i (sorted) =
    #   slot-base of its (core, blk, hl) group * P + rank within group
    grp_start = np.zeros(ngroups_keys + 1, dtype=np.int64)
    np.cumsum(cnt, out=grp_start[1:])
    rank = np.arange(len(key_s)) - grp_start[key_s]
    base = np.where(
        key_s % 2 == 0, blk_lo0[(key_s // 2) % nb], blk_hi0[(key_s // 2) % nb]
    )
    pos = base * P + rank  # within-core token position

    T = S_total * P
    gidx_np = np.zeros((n_cores, T), dtype=np.int16)
    dstl_np = np.full((n_cores, T), -1.0, dtype=np.float32)
    core_s = key_s // (2 * nb)
    gidx_np[core_s, pos] = gidx_s.astype(np.int16)
    dstl_np[core_s, pos] = dstl_s.astype(np.float32)

    plan = Plan()
    plan.n = n
    plan.n_cores = n_cores
    plan.npc = npc
    plan.nb = nb
    plan.npc_pad = npc_pad
    plan.S_lo = S_lo
    plan.S_hi = S_hi
    plan.blk_lo0 = blk_lo0
    plan.blk_hi0 = blk_hi0
    plan.chunks = chunks
    plan.S_total = S_total
    plan.max_chunk_slots = int(max(ns for (_, ns, _, _) in chunks))
    plan.dinv = dinv
    plan.nbA = nbA
    plan.hA = hA
    plan.hB = hB

    # device-ready arrays
    # gather idx: [128, T/16] int16, token j -> (j%16, j//16), replicated x8
    w = gidx_np.reshape(n_cores, -1, 16).transpose(0, 2, 1)  # [c, 16, T/16]
    plan.gidx = np.tile(w, (1, 8, 1)).copy()  # [c, 128, T/16]
    # dstlocal scalars: [128, S_total] bf16, token (s, p) -> [p, s]
    plan.dstl = (
        dstl_np.reshape(n_cores, S_total, P).transpose(0, 2, 1).astype(BF16).copy()
    )
    # dinv columns: [128, nb] f32 per core
    dpad = np.zeros((n_cores, npc_pad), dtype=np.float32)
    dpad[:, :npc] = dinv.reshape(n_cores, npc)
    plan.dinv_cols = dpad.reshape(n_cores, nb, P).transpose(0, 2, 1).copy()
    return plan


# ----------------------------------------------------------------------------
# device program
# ----------------------------------------------------------------------------

def build_program(plan, n_layers, d, use_collective=True, use_gather=True,
                  with_bias=True):
    nb, S_lo, S_hi = plan.nb, plan.S_lo, plan.S_hi
    npc, npc_pad, S_total = plan.npc, plan.npc_pad, plan.S_total
    n, n_cores = plan.n, plan.n_cores
    CS = plan.max_chunk_slots
    L = n_layers
    T16 = S_total * P // 16

    nc = bacc.Bacc("TRN2", target_bir_lowering=False, debug=False,
                   num_devices=n_cores)

    x0_in = nc.dram_tensor("x0sh", [npc_pad, d], F32, kind="ExternalInput")
    gidx_in = nc.dram_tensor("gidx", [P, T16], I16, kind="ExternalInput")
    dstl_in = nc.dram_tensor("dstl", [P, S_total], BF, kind="ExternalInput")
    dinv_in = nc.dram_tensor("dinvc", [P, nb], F32, kind="ExternalInput")
    w_in = nc.dram_tensor("wts", [L, d, d], F32, kind="ExternalInput")
    b_in = nc.dram_tensor("brep", [L, P, d], F32, kind="ExternalInput")
    iota_in = nc.dram_tensor("iota", [P, P], BF, kind="ExternalInput")
    id_in = nc.dram_tensor("ident", [P, P], F32, kind="ExternalInput")

    out_e = [
        nc.dram_tensor(f"out_e{l + 1}", [npc_pad, d], F32, kind="ExternalOutput")
        for l in range(L)
    ]
    out_tot = nc.dram_tensor("out_tot", [npc_pad, d], F32, kind="ExternalOutput")

    nbA, hA, hB = plan.nbA, plan.hA, plan.hB
    zka = nc.dram_tensor("zka", [hA, d], BF)
    zkb = nc.dram_tensor("zkb", [max(hB, 1), d], BF)
    zfull = [
        nc.dram_tensor(f"zfull{i}", [n, d], BF, addr_space="Shared")
        for i in range(2)
    ]
    rg = [list(range(n_cores))]

    with tile.TileContext(nc) as tc:
        with (
            tc.tile_pool(name="const", bufs=1) as cpool,
            tc.tile_pool(name="resident", bufs=1) as rpool,
            tc.tile_pool(name="gt", bufs=3) as gpool,
            tc.tile_pool(name="ht", bufs=3) as hpool,
            tc.tile_pool(name="work", bufs=6) as wpool,
            tc.tile_pool(name="zw", bufs=4) as zpool,
            tc.tile_pool(name="pa", bufs=3, space="PSUM") as pa_pool,
            tc.tile_pool(name="pt", bufs=2, space="PSUM") as pt_pool,
            tc.tile_pool(name="pz", bufs=2, space="PSUM") as pz_pool,
        ):
            # ---- constants / resident state ----
            nc.gpsimd.load_library(library_config.mlp)
            iota_sb = cpool.tile([P, P], BF)
            ident_sb = cpool.tile([P, P], F32)
            dstl_sb = cpool.tile([P, S_total], BF)
            dinv_sb = cpool.tile([P, nb], F32)
            gidx_sb = cpool.tile([P, T16], I16)
            w_sb = cpool.tile([P, L * d], F32)
            b_sb = cpool.tile([P, L * d], F32)
            nc.sync.dma_start(iota_sb[:], iota_in[:])
            nc.sync.dma_start(ident_sb[:], id_in[:])
            nc.sync.dma_start(dstl_sb[:], dstl_in[:])
            nc.sync.dma_start(dinv_sb[:], dinv_in[:])
            nc.sync.dma_start(gidx_sb[:], gidx_in[:])
            for l in range(L):
                nc.sync.dma_start(w_sb[:, l * d : (l + 1) * d], w_in[l, :, :])
                nc.sync.dma_start(b_sb[:, l * d : (l + 1) * d], b_in[l, :, :])

            x_state_a = rpool.tile([P, nb * d], F32, tag="x_stateA")
            x_state_b = rpool.tile([P, nb * d], F32, tag="x_stateB")
            x_ab = [x_state_a, x_state_b]
            tot_sb = rpool.tile([P, nb * d], F32, tag="tot_state")
            zself_sb = rpool.tile([P, nb * d], BF, tag="zself")
            for r in range(nb):
                nc.sync.dma_start(
                    x_ab[0][:, r * d : (r + 1) * d], x0_in[r * P : (r + 1) * P, :]
                )

            def z_block(l, x_src, r):
                """z row-block r for layer l: z = (dinv*x)@W_l -> zka/zkb,
                plus the self-loop term dinv*z kept in SBUF."""
                xs = wpool.tile([P, d], F32, tag="xs", name=f"xs{l}_{r}")
                nc.vector.tensor_scalar(
                    out=xs[:],
                    in0=x_src[:, r * d : (r + 1) * d],
                    scalar1=dinv_sb[:, r : r + 1],
                    scalar2=None,
                    op0=mybir.AluOpType.mult,
                )
                ptr = pt_pool.tile(
                    [P, P], F32, space="PSUM", tag="ptr", name=f"ptr{l}_{r}"
                )
                nc.tensor.transpose(out=ptr[:], in_=xs[:], identity=ident_sb[:])
                xT = wpool.tile([P, P], F32, tag="xT", name=f"xT{l}_{r}")
                nc.scalar.activation(
                    xT[:], ptr[:], mybir.ActivationFunctionType.Copy
                )
                pz = pz_pool.tile(
                    [P, d], F32, space="PSUM", tag="pz", name=f"pz{l}_{r}"
                )
                nc.tensor.matmul(
                    out=pz[:],
                    lhsT=xT[:],
                    rhs=w_sb[:, l * d : (l + 1) * d],
                    start=True,
                    stop=True,
                )
                zt = zpool.tile([P, d], BF, tag="zt", name=f"zt{l}_{r}")
                nc.vector.tensor_copy(out=zt[:], in_=pz[:])
                nc.vector.tensor_scalar(
                    out=zself_sb[:, r * d : (r + 1) * d],
                    in0=pz[:],
                    scalar1=dinv_sb[:, r : r + 1],
                    scalar2=None,
                    op0=mybir.AluOpType.mult,
                )
                if r < nbA:
                    nc.sync.dma_start(zka[r * P : (r + 1) * P, :], zt[:, :])
                else:
                    rows = min(P, npc - r * P)
                    rb = (r - nbA) * P
                    nc.sync.dma_start(zkb[rb : rb + rows, :], zt[:rows, :])

            def ag_a(zf_dst):
                if use_collective:
                    nc.gpsimd.collective_compute(
                        "AllGather",
                        mybir.AluOpType.bypass,
                        ins=[zka[:, :]],
                        outs=[zf_dst[: n_cores * hA, :]],
                        replica_groups=rg,
                    )
                else:
                    nc.sync.dma_start(zf_dst[:hA, :], zka[:, :])

            def ag_b(zf_dst):
                if hB <= 0:
                    return
                if use_collective:
                    nc.gpsimd.collective_compute(
                        "AllGather",
                        mybir.AluOpType.bypass,
                        ins=[zkb[:, :]],
                        outs=[zf_dst[n_cores * hA :, :]],
                        replica_groups=rg,
                    )
                else:
                    nc.sync.dma_start(zf_dst[hA:npc, :], zkb[:, :])

            # prologue: layer-0 z table from x0
            for r in range(nbA):
                z_block(0, x_ab[0], r)
            ag_a(zfull[0])
            for r in range(nbA, nb):
                z_block(0, x_ab[0], r)
            ag_b(zfull[0])

            for l in range(L):
                zf = zfull[l % 2]
                zf_next = zfull[(l + 1) % 2]
                x_prev = x_ab[l % 2]
                x_next = x_ab[(l + 1) % 2]
                blocks_done = 0
                # ---- edge phase ----
                # gather chunks + H chunks, then PE aggregation per block
                gtiles = {}
                htiles = {}
                for ci, (c0, ns, is_hi, rs) in enumerate(plan.chunks):
                    gt = gpool.tile([P, CS, d], BF, tag="gt")
                    hi_base = LOW_LIM if n > LOW_LIM else 0
                    src_view = zf[hi_base:, :] if is_hi else zf[:, :]
                    if use_gather:
                        nc.gpsimd.dma_gather(
                            out_ap=gt[:, :ns, :],
                            in_ap=src_view,
                            idxs_ap=gidx_sb[:, c0 * 8 : (c0 + ns) * 8],
                            num_idxs=ns * P,
                            num_idxs_reg=ns * P,
                            elem_size=d,
                            single_packet=False,
                        )
                    else:
                        nc.sync.dma_start(
                            gt[:, :ns, :],
                            zf[: ns * P, :].rearrange(
                                "(s p) c -> p s c", p=P
                            ),
                        )
                    ht = hpool.tile([P, CS * d], BF, tag="ht")
                    nc.vector.tensor_tensor(
                        out=ht[:, : ns * d].rearrange("p (s c) -> p s c", s=ns),
                        in0=iota_sb[:, None, :].to_broadcast([P, ns, P]),
                        in1=dstl_sb[:, c0 : c0 + ns, None].to_broadcast([P, ns, P]),
                        op=mybir.AluOpType.is_equal,
                    )
                    gtiles[ci] = gt
                    htiles[ci] = ht
                    # after both chunks of a block-group are in flight, do PE
                    if is_hi:
                        for r in rs:
                            pacc = pa_pool.tile([P, d], F32, space="PSUM")
                            work = []
                            for (s0, cnt_s) in (
                                (plan.blk_lo0[r], S_lo[r]),
                                (plan.blk_hi0[r], S_hi[r]),
                            ):
                                for s in range(s0, s0 + cnt_s):
                                    work.append(s)
                            for wi, s in enumerate(work):
                                # find chunk containing slot s
                                cj = next(
                                    j
                                    for j, (d0, dn, _, _) in enumerate(plan.chunks)
                                    if d0 <= s < d0 + dn
                                )
                                gt_j = gtiles[cj]
                                ht_j = htiles[cj]
                                so = s - plan.chunks[cj][0]
                                nc.tensor.matmul(
                                    out=pacc[:],
                                    lhsT=ht_j[:, so * d : (so + 1) * d],
                                    rhs=gt_j[:, so, :],
                                    start=(wi == 0),
                                    stop=(wi == len(work) - 1),
                                )
                            # ---- epilogue for block r ----
                            # e = dinv*pacc + dinv*z_own + b, written in place
                            # into the next-layer x state
                            ecol = x_next[:, r * d : (r + 1) * d]
                            zscol = zself_sb[:, r * d : (r + 1) * d]
                            nc.scalar.activation(
                                ecol,
                                pacc[:],
                                mybir.ActivationFunctionType.Copy,
                                scale=dinv_sb[:, r : r + 1],
                            )
                            nc.vector.tensor_tensor(
                                out=ecol, in0=ecol, in1=zscol,
                                op=mybir.AluOpType.add,
                            )
                            if with_bias:
                                nc.vector.tensor_tensor(
                                    out=ecol,
                                    in0=ecol,
                                    in1=b_sb[:, l * d : (l + 1) * d],
                                    op=mybir.AluOpType.add,
                                )
                            nc.sync.dma_start(
                                out_e[l][r * P : (r + 1) * P, :], ecol
                            )
                            xcol = x_prev[:, r * d : (r + 1) * d]
                            tcol = tot_sb[:, r * d : (r + 1) * d]
                            if l == 0:
                                nc.vector.tensor_tensor(
                                    out=tcol, in0=xcol, in1=ecol,
                                    op=mybir.AluOpType.add,
                                )
                            elif l < L - 1:
                                nc.vector.tensor_tensor(
                                    out=tcol, in0=tcol, in1=ecol,
                                    op=mybir.AluOpType.add,
                                )
                            else:
                                tf = wpool.tile([P, d], F32, tag="tf")
                                nc.vector.tensor_tensor(
                                    out=tf[:], in0=tcol, in1=ecol,
                                    op=mybir.AluOpType.add,
                                )
                                nc.sync.dma_start(
                                    out_tot[r * P : (r + 1) * P, :], tf[:]
                                )
                            # next layer's z for this block, right behind the
                            # epilogue so its half-A AllGather can fire while
                            # this layer's remaining gathers still run
                            if l < L - 1:
                                z_block(l + 1, x_next, r)
                                blocks_done += 1
                                if blocks_done == nbA:
                                    ag_a(zf_next)
                if l < L - 1:
                    ag_b(zf_next)
    nc.compile()
    return nc


# ----------------------------------------------------------------------------
# top-level entry
# ----------------------------------------------------------------------------

def make_in_maps(plan, item_emb, weights, biases, n_layers, d):
    n, n_cores, npc, npc_pad = plan.n, plan.n_cores, plan.npc, plan.npc_pad
    x0 = np.asarray(item_emb, dtype=np.float32)[-n:]
    iota_np = np.tile(np.arange(P, dtype=np.float32), (P, 1)).astype(BF16)
    ident_np = np.eye(P, dtype=np.float32)
    w_np = np.asarray(weights, dtype=np.float32)
    b_np = np.asarray(biases, dtype=np.float32)
    b_rep = np.tile(b_np[:, None, :], (1, P, 1)).astype(np.float32)

    in_maps = []
    for c in range(n_cores):
        x0sh = np.zeros((npc_pad, d), dtype=np.float32)
        x0sh[:npc] = x0[c * npc : (c + 1) * npc]
        in_maps.append(
            {
                "x0sh": x0sh,
                "gidx": plan.gidx[c],
                "dstl": plan.dstl[c],
                "dinvc": plan.dinv_cols[c],
                "wts": w_np,
                "brep": b_rep,
                "iota": iota_np,
                "ident": ident_np,
            }
        )
    return in_maps


def assemble_outputs(plan, results, item_emb, n_layers):
    n, n_cores, npc = plan.n, plan.n_cores, plan.npc
    x0 = np.asarray(item_emb, dtype=np.float32)[-n:]
    outs = []
    tot = np.concatenate([results[c]["out_tot"][:npc] for c in range(n_cores)])
    outs.append(tot)
    outs.append(x0)
    for l in range(n_layers):
        e = np.concatenate(
            [results[c][f"out_e{l + 1}"][:npc] for c in range(n_cores)]
        )
        outs.append(e)
    return tuple(outs)


_CACHE = {}


def kernel(item_emb, weights, biases, edge_index, item_nums):
    from concourse.bass_utils import run_bass_kernel_spmd

    n = int(item_nums)
    L, d, _ = np.asarray(weights).shape
    n_cores = 8

    plan = build_plan(np.asarray(edge_index), n, n_cores)
    nc = build_program(plan, L, d, with_bias=bool(np.any(np.asarray(biases))))
    in_maps = make_in_maps(plan, item_emb, weights, biases, L, d)
    res = run_bass_kernel_spmd(nc, in_maps, list(range(n_cores)))
    return assemble_outputs(plan, res.results, item_emb, L)


# revision 34
# speedup vs baseline: 1.0300x; 1.0300x over previous
"""3-layer GCN encoder (PyG GCNConv semantics) on 8 Trainium2 NeuronCores.

Strategy (dst-sharded message passing):
  - Nodes are 1D-partitioned across the 8 cores (node n -> core n // (N/8)).
  - Per layer l:  z = (dinv * x) @ W_l   computed shard-wise in fp32 on PE,
    rounded to bf16, AllGathered into a full [N, D] bf16 table in DRAM.
    (norm factorizes: out[i] = dinv[i] * sum_e dinv[src_e] * (x@W)[src_e],
     so no per-edge scaling is needed.)
  - Each core owns the edges whose dst lands in its shard. Edge messages are
    fetched with GPSIMD dma_gather (HBM->SBUF, 256B rows by int16 index; srcs
    >= 32768 are gathered through a +32768 base view).  Aggregation is done
    on the PE: for each 128-token slot, a one-hot matrix H[t, dstlocal_t]
    (built on DVE via iota==dstlocal) is matmul'ed with the gathered tokens
    and accumulated in PSUM per 128-node block -> exact fp32 segment sum.
  - Epilogue: e = dinv * agg + b (fp32), written out per shard; running
    total and next-layer x stay SBUF-resident.

Host side: edge bucketing by (core, block, low/high src), per-block padding
to 128-token slots with (src=0, dstlocal=-1) dummies (H row is all-zero so
they contribute nothing), index/one-hot-scalar array packing, and final
unshard (concat + slice).
"""

import math

import numpy as np
import ml_dtypes

from concourse import bass, bacc, mybir, library_config
import concourse.tile as tile

BF16 = ml_dtypes.bfloat16
P = 128
LOW_LIM = 32768
F32 = mybir.dt.float32
BF = mybir.dt.bfloat16
I16 = mybir.dt.int16


# ----------------------------------------------------------------------------
# host-side preprocessing
# ----------------------------------------------------------------------------

class Plan:
    """Static (core-invariant) program structure + per-core packed arrays."""
    pass


def build_plan(edge_index, n, n_cores, group_blocks=7, max_chunk_slots=80):
    src_e = np.asarray(edge_index[0], dtype=np.int64)
    dst_e = np.asarray(edge_index[1], dtype=np.int64)
    # self-loops are handled analytically on-device (dinv * z term), so the
    # token stream only carries the real edges; degree still counts them.
    src = src_e
    dst = dst_e

    deg = (np.bincount(dst, minlength=n) + 1).astype(np.float64)
    dinv = (1.0 / np.sqrt(deg)).astype(np.float32)

    assert n % n_cores == 0
    npc = n // n_cores
    nb = math.ceil(npc / P)
    npc_pad = nb * P

    # z-table order: per-layer AllGather is split in two so the big first
    # part can overlap the previous layer's gathers. Blocks [0, nbA) of every
    # core land in table half A (rank-major), the last group's blocks in
    # half B. Gather indices use this table position, not the node id.
    nbA = max(nb - group_blocks, 1) if nb > group_blocks else nb
    hA = min(nbA * P, npc)
    hB = npc - hA

    core = dst // npc
    blk = (dst % npc) // P
    dstl = (dst % npc) % P
    src_core = src // npc
    src_loc = src % npc
    tpos = np.where(
        src_loc < hA,
        src_core * hA + src_loc,
        n_cores * hA + src_core * hB + (src_loc - hA),
    )
    low = tpos < LOW_LIM
    gidx_val = np.where(low, tpos, tpos - LOW_LIM)

    # stable-sort edges by (core, blk, high?) -> contiguous groups
    key = (core * nb + blk) * 2 + (~low).astype(np.int64)
    order = np.argsort(key, kind="stable")
    key_s = key[order]
    gidx_s = gidx_val[order]
    dstl_s = dstl[order]

    ngroups_keys = n_cores * nb * 2
    cnt = np.bincount(key_s, minlength=ngroups_keys)
    cnt_lo = cnt[0::2].reshape(n_cores, nb)
    cnt_hi = cnt[1::2].reshape(n_cores, nb)

    # slots per block index (max over cores) -> core-invariant structure
    S_lo = np.maximum(1, np.ceil(cnt_lo / P).astype(np.int64).max(axis=0))
    S_hi = np.maximum(1, np.ceil(cnt_hi / P).astype(np.int64).max(axis=0))

    # slot layout: per group of blocks: [lo slots of blocks][hi slots of blocks]
    G = group_blocks
    ngrp = math.ceil(nb / G)
    blk_lo0 = np.zeros(nb, dtype=np.int64)
    blk_hi0 = np.zeros(nb, dtype=np.int64)
    chunks = []  # (slot0, nslots, is_high, [block ids])
    s_cur = 0
    for g in range(ngrp):
        rs = list(range(g * G, min(nb, (g + 1) * G)))
        c0 = s_cur
        for r in rs:
            blk_lo0[r] = s_cur
            s_cur += S_lo[r]
        # split long runs into <= max_chunk_slots chunks
        chunks.append((c0, s_cur - c0, False, rs))
        c0 = s_cur
        for r in rs:
            blk_hi0[r] = s_cur
            s_cur += S_hi[r]
        chunks.append((c0, s_cur - c0, True, rs))
    S_total = s_cur

    # split oversize chunks (keeps gather tiles bounded)
    chunks2 = []
    for (c0, ns, hi, rs) in chunks:
        while ns > max_chunk_slots:
            chunks2.append((c0, max_chunk_slots, hi, rs))
            c0 += max_chunk_slots
            ns -= max_chunk_slots
        if ns > 0:
            chunks2.append((c0, ns, hi, rs))
    chunks = chunks2

    # pack tokens: position of edge i (sorted) =
    #   slot-base of its (core, blk, hl) group * P + rank within group
    grp_start = np.zeros(ngroups_keys + 1, dtype=np.int64)
    np.cumsum(cnt, out=grp_start[1:])
    rank = np.arange(len(key_s)) - grp_start[key_s]
    base = np.where(
        key_s % 2 == 0, blk_lo0[(key_s // 2) % nb], blk_hi0[(key_s // 2) % nb]
    )
    pos = base * P + rank  # within-core token position

    T = S_total * P
    gidx_np = np.zeros((n_cores, T), dtype=np.int16)
    dstl_np = np.full((n_cores, T), -1.0, dtype=np.float32)
    core_s = key_s // (2 * nb)
    gidx_np[core_s, pos] = gidx_s.astype(np.int16)
    dstl_np[core_s, pos] = dstl_s.astype(np.float32)

    plan = Plan()
    plan.n = n
    plan.n_cores = n_cores
    plan.npc = npc
    plan.nb = nb
    plan.npc_pad = npc_pad
    plan.S_lo = S_lo
    plan.S_hi = S_hi
    plan.blk_lo0 = blk_lo0
    plan.blk_hi0 = blk_hi0
    plan.chunks = chunks
    plan.S_total = S_total
    plan.max_chunk_slots = int(max(ns for (_, ns, _, _) in chunks))
    plan.dinv = dinv
    plan.nbA = nbA
    plan.hA = hA
    plan.hB = hB

    # device-ready arrays
    # gather idx: [128, T/16] int16, token j -> (j%16, j//16), replicated x8
    w = gidx_np.reshape(n_cores, -1, 16).transpose(0, 2, 1)  # [c, 16, T/16]
    plan.gidx = np.tile(w, (1, 8, 1)).copy()  # [c, 128, T/16]
    # dstlocal scalars: [128, S_total] bf16, token (s, p) -> [p, s]
    plan.dstl = (
        dstl_np.reshape(n_cores, S_total, P).transpose(0, 2, 1).astype(BF16).copy()
    )
    # dinv columns: [128, nb] f32 per core
    dpad = np.zeros((n_cores, npc_pad), dtype=np.float32)
    dpad[:, :npc] = dinv.reshape(n_cores, npc)
    plan.dinv_cols = dpad.reshape(n_cores, nb, P).transpose(0, 2, 1).copy()
    return plan


# ----------------------------------------------------------------------------
# device program
# ----------------------------------------------------------------------------

def build_program(plan, n_layers, d, use_collective=True, use_gather=True,
                  with_bias=True):
    nb, S_lo, S_hi = plan.nb, plan.S_lo, plan.S_hi
    npc, npc_pad, S_total = plan.npc, plan.npc_pad, plan.S_total
    n, n_cores = plan.n, plan.n_cores
    CS = plan.max_chunk_slots
    L = n_layers
    T16 = S_total * P // 16

    nc = bacc.Bacc("TRN2", target_bir_lowering=False, debug=False,
                   num_devices=n_cores)

    x0_in = nc.dram_tensor("x0sh", [npc_pad, d], F32, kind="ExternalInput")
    gidx_in = nc.dram_tensor("gidx", [P, T16], I16, kind="ExternalInput")
    dstl_in = nc.dram_tensor("dstl", [P, S_total], BF, kind="ExternalInput")
    dinv_in = nc.dram_tensor("dinvc", [P, nb], F32, kind="ExternalInput")
    w_in = nc.dram_tensor("wts", [L, d, d], F32, kind="ExternalInput")
    b_in = nc.dram_tensor("brep", [L, P, d], F32, kind="ExternalInput")
    iota_in = nc.dram_tensor("iota", [P, P], BF, kind="ExternalInput")
    id_in = nc.dram_tensor("ident", [P, P], F32, kind="ExternalInput")

    out_e = [
        nc.dram_tensor(f"out_e{l + 1}", [npc_pad, d], F32, kind="ExternalOutput")
        for l in range(L)
    ]
    out_tot = nc.dram_tensor("out_tot", [npc_pad, d], F32, kind="ExternalOutput")

    nbA, hA, hB = plan.nbA, plan.hA, plan.hB
    zka = nc.dram_tensor("zka", [hA, d], BF)
    zkb = nc.dram_tensor("zkb", [max(hB, 1), d], BF)
    zfull = [
        nc.dram_tensor(f"zfull{i}", [n, d], BF, addr_space="Shared")
        for i in range(2)
    ]
    rg = [list(range(n_cores))]

    with tile.TileContext(nc) as tc:
        with (
            tc.tile_pool(name="const", bufs=1) as cpool,
            tc.tile_pool(name="resident", bufs=1) as rpool,
            tc.tile_pool(name="gt", bufs=3) as gpool,
            tc.tile_pool(name="ht", bufs=3) as hpool,
            tc.tile_pool(name="work", bufs=6) as wpool,
            tc.tile_pool(name="zw", bufs=4) as zpool,
            tc.tile_pool(name="pa", bufs=3, space="PSUM") as pa_pool,
            tc.tile_pool(name="pt", bufs=2, space="PSUM") as pt_pool,
            tc.tile_pool(name="pz", bufs=2, space="PSUM") as pz_pool,
        ):
            # ---- constants / resident state ----
            nc.gpsimd.load_library(library_config.mlp)
            iota_sb = cpool.tile([P, P], BF)
            ident_sb = cpool.tile([P, P], F32)
            dstl_sb = cpool.tile([P, S_total], BF)
            dinv_sb = cpool.tile([P, nb], F32)
            gidx_sb = cpool.tile([P, T16], I16)
            w_sb = cpool.tile([P, L * d], F32)
            b_sb = cpool.tile([P, L * d], F32)
            nc.sync.dma_start(iota_sb[:], iota_in[:])
            nc.sync.dma_start(ident_sb[:], id_in[:])
            nc.sync.dma_start(dstl_sb[:], dstl_in[:])
            nc.sync.dma_start(dinv_sb[:], dinv_in[:])
            nc.sync.dma_start(gidx_sb[:], gidx_in[:])
            for l in range(L):
                nc.sync.dma_start(w_sb[:, l * d : (l + 1) * d], w_in[l, :, :])
                nc.sync.dma_start(b_sb[:, l * d : (l + 1) * d], b_in[l, :, :])

            x_state_a = rpool.tile([P, nb * d], F32, tag="x_stateA")
            x_state_b = rpool.tile([P, nb * d], F32, tag="x_stateB")
            x_ab = [x_state_a, x_state_b]
            tot_sb = rpool.tile([P, nb * d], F32, tag="tot_state")
            zself_sb = rpool.tile([P, nb * d], BF, tag="zself")
            for r in range(nb):
                nc.sync.dma_start(
                    x_ab[0][:, r * d : (r + 1) * d], x0_in[r * P : (r + 1) * P, :]
                )

            def z_block(l, x_src, r):
                """z row-block r for layer l: z = (dinv*x)@W_l -> zka/zkb,
                plus the self-loop term dinv*z kept in SBUF."""
                xs = wpool.tile([P, d], F32, tag="xs", name=f"xs{l}_{r}")
                nc.vector.tensor_scalar(
                    out=xs[:],
                    in0=x_src[:, r * d : (r + 1) * d],
                    scalar1=dinv_sb[:, r : r + 1],
                    scalar2=None,
                    op0=mybir.AluOpType.mult,
                )
                ptr = pt_pool.tile(
                    [P, P], F32, space="PSUM", tag="ptr", name=f"ptr{l}_{r}"
                )
                nc.tensor.transpose(out=ptr[:], in_=xs[:], identity=ident_sb[:])
                xT = wpool.tile([P, P], F32, tag="xT", name=f"xT{l}_{r}")
                nc.scalar.activation(
                    xT[:], ptr[:], mybir.ActivationFunctionType.Copy
                )
                pz = pz_pool.tile(
                    [P, d], F32, space="PSUM", tag="pz", name=f"pz{l}_{r}"
                )
                nc.tensor.matmul(
                    out=pz[:],
                    lhsT=xT[:],
                    rhs=w_sb[:, l * d : (l + 1) * d],
                    start=True,
                    stop=True,
                )
                zt = zpool.tile([P, d], BF, tag="zt", name=f"zt{l}_{r}")
                nc.vector.tensor_copy(out=zt[:], in_=pz[:])
                nc.vector.tensor_scalar(
                    out=zself_sb[:, r * d : (r + 1) * d],
                    in0=pz[:],
                    scalar1=dinv_sb[:, r : r + 1],
                    scalar2=None,
                    op0=mybir.AluOpType.mult,
                )
                if r < nbA:
                    nc.sync.dma_start(zka[r * P : (r + 1) * P, :], zt[:, :])
                else:
                    rows = min(P, npc - r * P)
                    rb = (r - nbA) * P
                    nc.sync.dma_start(zkb[rb : rb + rows, :], zt[:rows, :])

            def ag_a(zf_dst):
                if use_collective:
                    nc.gpsimd.collective_compute(
                        "AllGather",
                        mybir.AluOpType.bypass,
                        ins=[zka[:, :]],
                        outs=[zf_dst[: n_cores * hA, :]],
                        replica_groups=rg,
                    )
                else:
                    nc.sync.dma_start(zf_dst[:hA, :], zka[:, :])

            def ag_b(zf_dst):
                if hB <= 0:
                    return
                if use_collective:
                    nc.gpsimd.collective_compute(
                        "AllGather",
                        mybir.AluOpType.bypass,
                        ins=[zkb[:, :]],
                        outs=[zf_dst[n_cores * hA :, :]],
                        replica_groups=rg,
                    )
                else:
                    nc.sync.dma_start(zf_dst[hA:npc, :], zkb[:, :])

            # prologue: layer-0 z table from x0
            for r in range(nbA):
                z_block(0, x_ab[0], r)
            ag_a(zfull[0])
            for r in range(nbA, nb):
                z_block(0, x_ab[0], r)
            ag_b(zfull[0])

            for l in range(L):
                zf = zfull[l % 2]
                zf_next = zfull[(l + 1) % 2]
                x_prev = x_ab[l % 2]
                x_next = x_ab[(l + 1) % 2]
                blocks_done = 0
                ag_a_done = False
                # ---- edge phase ----
                # gather chunks + H chunks, then PE aggregation per block
                gtiles = {}
                htiles = {}
                for ci, (c0, ns, is_hi, rs) in enumerate(plan.chunks):
                    gt = gpool.tile([P, CS, d], BF, tag="gt")
                    hi_base = LOW_LIM if n > LOW_LIM else 0
                    src_view = zf[hi_base:, :] if is_hi else zf[:, :]
                    if use_gather:
                        nc.gpsimd.dma_gather(
                            out_ap=gt[:, :ns, :],
                            in_ap=src_view,
                            idxs_ap=gidx_sb[:, c0 * 8 : (c0 + ns) * 8],
                            num_idxs=ns * P,
                            num_idxs_reg=ns * P,
                            elem_size=d,
                            single_packet=False,
                        )
                    else:
                        nc.sync.dma_start(
                            gt[:, :ns, :],
                            zf[: ns * P, :].rearrange(
                                "(s p) c -> p s c", p=P
                            ),
                        )
                    ht = hpool.tile([P, CS * d], BF, tag="ht")
                    nc.vector.tensor_tensor(
                        out=ht[:, : ns * d].rearrange("p (s c) -> p s c", s=ns),
                        in0=iota_sb[:, None, :].to_broadcast([P, ns, P]),
                        in1=dstl_sb[:, c0 : c0 + ns, None].to_broadcast([P, ns, P]),
                        op=mybir.AluOpType.is_equal,
                    )
                    gtiles[ci] = gt
                    htiles[ci] = ht
                    # half-A AllGather of the NEXT layer's z table: its inputs
                    # (z blocks 0..nbA-1) are ready once groups 0..ngrp-2 are
                    # through, i.e. before the last lo-chunk's desc-gen ends —
                    # emitting it here lets it overlap the final hi chunk
                    # without stalling the GPSIMD stream.
                    if (
                        l < L - 1
                        and ci == len(plan.chunks) - 2
                        and blocks_done >= nbA
                        and not ag_a_done
                    ):
                        ag_a(zf_next)
                        ag_a_done = True
                    # after both chunks of a block-group are in flight, do PE
                    if is_hi:
                        for r in rs:
                            pacc = pa_pool.tile([P, d], F32, space="PSUM")
                            work = []
                            for (s0, cnt_s) in (
                                (plan.blk_lo0[r], S_lo[r]),
                                (plan.blk_hi0[r], S_hi[r]),
                            ):
                                for s in range(s0, s0 + cnt_s):
                                    work.append(s)
                            for wi, s in enumerate(work):
                                # find chunk containing slot s
                                cj = next(
                                    j
                                    for j, (d0, dn, _, _) in enumerate(plan.chunks)
                                    if d0 <= s < d0 + dn
                                )
                                gt_j = gtiles[cj]
                                ht_j = htiles[cj]
                                so = s - plan.chunks[cj][0]
                                nc.tensor.matmul(
                                    out=pacc[:],
                                    lhsT=ht_j[:, so * d : (so + 1) * d],
                                    rhs=gt_j[:, so, :],
                                    start=(wi == 0),
                                    stop=(wi == len(work) - 1),
                                )
                            # ---- epilogue for block r ----
                            # e = dinv*pacc + dinv*z_own + b, written in place
                            # into the next-layer x state
                            ecol = x_next[:, r * d : (r + 1) * d]
                            zscol = zself_sb[:, r * d : (r + 1) * d]
                            nc.scalar.activation(
                                ecol,
                                pacc[:],
                                mybir.ActivationFunctionType.Copy,
                                scale=dinv_sb[:, r : r + 1],
                            )
                            nc.vector.tensor_tensor(
                                out=ecol, in0=ecol, in1=zscol,
                                op=mybir.AluOpType.add,
                            )
                            if with_bias:
                                nc.vector.tensor_tensor(
                                    out=ecol,
                                    in0=ecol,
                                    in1=b_sb[:, l * d : (l + 1) * d],
                                    op=mybir.AluOpType.add,
                                )
                            nc.sync.dma_start(
                                out_e[l][r * P : (r + 1) * P, :], ecol
                            )
                            xcol = x_prev[:, r * d : (r + 1) * d]
                            tcol = tot_sb[:, r * d : (r + 1) * d]
                            if l == 0:
                                nc.vector.tensor_tensor(
                                    out=tcol, in0=xcol, in1=ecol,
                                    op=mybir.AluOpType.add,
                                )
                            elif l < L - 1:
                                nc.vector.tensor_tensor(
                                    out=tcol, in0=tcol, in1=ecol,
                                    op=mybir.AluOpType.add,
                                )
                            else:
                                tf = wpool.tile([P, d], F32, tag="tf")
                                nc.vector.tensor_tensor(
                                    out=tf[:], in0=tcol, in1=ecol,
                                    op=mybir.AluOpType.add,
                                )
                                nc.sync.dma_start(
                                    out_tot[r * P : (r + 1) * P, :], tf[:]
                                )
                            # next layer's z for this block, right behind the
                            # epilogue so its half-A AllGather can fire while
                            # this layer's remaining gathers still run
                            if l < L - 1:
                                z_block(l + 1, x_next, r)
                                blocks_done += 1
                if l < L - 1:
                    if not ag_a_done:
                        ag_a(zf_next)
                    ag_b(zf_next)
    nc.compile()
    return nc


# ----------------------------------------------------------------------------
# top-level entry
# ----------------------------------------------------------------------------

def make_in_maps(plan, item_emb, weights, biases, n_layers, d):
    n, n_cores, npc, npc_pad = plan.n, plan.n_cores, plan.npc, plan.npc_pad
    x0 = np.asarray(item_emb, dtype=np.float32)[-n:]
    iota_np = np.tile(np.arange(P, dtype=np.float32), (P, 1)).astype(BF16)
    ident_np = np.eye(P, dtype=np.float32)
    w_np = np.asarray(weights, dtype=np.float32)
    b_np = np.asarray(biases, dtype=np.float32)
    b_rep = np.tile(b_np[:, None, :], (1, P, 1)).astype(np.float32)

    in_maps = []
    for c in range(n_cores):
        x0sh = np.zeros((npc_pad, d), dtype=np.float32)
        x0sh[:npc] = x0[c * npc : (c + 1) * npc]
        in_maps.append(
            {
                "x0sh": x0sh,
                "gidx": plan.gidx[c],
                "dstl": plan.dstl[c],
                "dinvc": plan.dinv_cols[c],
                "wts": w_np,
                "brep": b_rep,
                "iota": iota_np,
                "ident": ident_np,
            }
        )
    return in_maps


def assemble_outputs(plan, results, item_emb, n_layers):
    n, n_cores, npc = plan.n, plan.n_cores, plan.npc
    x0 = np.asarray(item_emb, dtype=np.float32)[-n:]
    outs = []
    tot = np.concatenate([results[c]["out_tot"][:npc] for c in range(n_cores)])
    outs.append(tot)
    outs.append(x0)
    for l in range(n_layers):
        e = np.concatenate(
            [results[c][f"out_e{l + 1}"][:npc] for c in range(n_cores)]
        )
        outs.append(e)
    return tuple(outs)


_CACHE = {}


def kernel(item_emb, weights, biases, edge_index, item_nums):
    from concourse.bass_utils import run_bass_kernel_spmd

    n = int(item_nums)
    L, d, _ = np.asarray(weights).shape
    n_cores = 8

    plan = build_plan(np.asarray(edge_index), n, n_cores)
    nc = build_program(plan, L, d, with_bias=bool(np.any(np.asarray(biases))))
    in_maps = make_in_maps(plan, item_emb, weights, biases, L, d)
    res = run_bass_kernel_spmd(nc, in_maps, list(range(n_cores)))
    return assemble_outputs(plan, res.results, item_emb, L)
